# revision 1
# baseline (speedup 1.0000x reference)
"""Trainium2 Bass kernel for nn_AttentionHead_6365141532793.

Computes (per batch b):
    q = query @ Wq.T + bq ; k = key @ Wq.T + bq ; v = value @ Wq.T + bq
    out = softmax(q @ k.T / sqrt(D)) @ v

Sharding: 8 cores = 4 batches x 2 query-slabs (2048 rows each). Each core
holds the full key/value of its batch, so softmax rows are complete per
core and no collectives are needed.

Per-core algorithm (matmuls in bf16 on the TensorEngine, f32 PSUM accum):
  - load Wq / q-slab / key as f32, transpose 128x128 blocks on the
    TensorEngine (transpose-mode matmul with identity), cast to bf16 on
    the PSUM->SBUF copy: WqT[d, o], qT[d, m], kT[d, n]
  - load value, cast bf16, keep natural v[n, d]
  - q_projT[o, m] = WqT.T @ qT  (+bq via per-partition scalar add)
  - k_projT[o, n] = WqT.T @ kT  (+bq)
  - for each m-super (512 query rows):
      for each key chunk (128 rows):
        scoresT[n, m] = k_projT_chunk.T @ q_projT_super   (PSUM)
        pT = exp(scoresT / 16)                            (ScalarE, bf16)
        UT[d, m]   += v_chunk.T @ pT                      (PSUM accum)
        s[1, m]    += ones.T @ pT                         (row sums)
      recip = 1/s; broadcast to 128 partitions via SBUF->SBUF DMA
      ut_sb = UT * recip (normalize while casting bf16)
      out[m, o] = ut_sb.T @ WqT + bq   (projection of U = P@V applied
      after attention: P @ (V Wq^T + 1 bq^T) = (P V) Wq^T + s bq^T)
No max-subtraction in softmax: scores/16 are O(1) for this distribution,
exp is safe in fp32/bf16 and softmax is shift-invariant anyway.
"""

import contextlib
import dataclasses

import numpy as np

B, S, D = 4, 4096, 256
NCORES = 8
NQ = S // 2          # query rows per core
P = 128
DC = D // P          # 2 chunks of the d/o dimension
MSUP = 512           # m-super: moving free dim per score matmul
GN = 1               # key chunks per exp batch (psum tile = GN banks)
SCALE = 1.0 / 16.0   # 1/sqrt(D)

_CACHE = {}


def _build(NQ=NQ, S=S, num_devices=NCORES):
    import concourse.mybir as mybir
    import concourse.tile as tile
    import concourse.bass as bass
    from concourse import bacc
    from concourse.masks import make_identity

    f32 = mybir.dt.float32
    bf16 = mybir.dt.bfloat16
    FT = mybir.ActivationFunctionType
    ALU = mybir.AluOpType

    NSUP = NQ // MSUP
    NKC = S // P

    nc = bacc.Bacc("TRN2", target_bir_lowering=False, debug=False,
                   num_devices=num_devices)
    q_ext = nc.dram_tensor("query", [NQ, D], f32, kind="ExternalInput").ap()
    k_ext = nc.dram_tensor("key", [S, D], f32, kind="ExternalInput").ap()
    v_ext = nc.dram_tensor("value", [S, D], f32, kind="ExternalInput").ap()
    w_ext = nc.dram_tensor("Wq", [D, D], f32, kind="ExternalInput").ap()
    b_ext = nc.dram_tensor("bq", [D], f32, kind="ExternalInput").ap()
    o_ext = nc.dram_tensor("out", [NQ, D], f32, kind="ExternalOutput").ap()

    with contextlib.ExitStack() as ctx:
        tc = ctx.enter_context(tile.TileContext(nc))
        singles = ctx.enter_context(tc.tile_pool(name="singles", bufs=1))
        fstage = ctx.enter_context(tc.tile_pool(name="fstage", bufs=3))
        ptpool = ctx.enter_context(tc.tile_pool(name="ptpool", bufs=3))
        utsbp = ctx.enter_context(tc.tile_pool(name="utsbp", bufs=2))
        rsbp = ctx.enter_context(tc.tile_pool(name="rsbp", bufs=2))
        osbp = ctx.enter_context(tc.tile_pool(name="osbp", bufs=3))
        ps_st = ctx.enter_context(tc.tile_pool(name="ps_st", bufs=2, space="PSUM"))
        ps_ut = ctx.enter_context(tc.tile_pool(name="ps_ut", bufs=1, space="PSUM"))
        ps_epi = ctx.enter_context(tc.tile_pool(name="ps_epi", bufs=2, space="PSUM"))

        # ---------------- constants ----------------
        ident = singles.tile([P, P], f32, tag="ident")
        make_identity(nc, ident)
        # full ones matrix: rowsum matmul with M=128 -> s replicated on all
        # 128 partitions (avoids M=1 matmuls and partition-broadcasts)
        ones_mat = singles.tile([P, P], bf16, tag="ones_mat")
        nc.vector.memset(ones_mat, 1.0)
        # bq_pc[p, oc] = bq[oc*P + p] (per-partition bias for projT layouts)
        bq_pc = singles.tile([P, DC], f32, tag="bq_pc")
        nc.sync.dma_start(out=bq_pc, in_=b_ext.rearrange("(c p) -> p c", p=P))
        # bq_full[p, o] = bq[o] -- broadcast across partitions (DMA step-0)
        bq_full = singles.tile([P, D], f32, tag="bq_full")
        b_bc = dataclasses.replace(
            b_ext, ap=[[0, P]] + [list(c) for c in b_ext.ap])
        nc.sync.dma_start(out=bq_full, in_=b_bc)

        # wqT[p, dc, o] = Wq[o, dc*P + p]  (PE transpose of f32, cast bf16)
        wq_f = singles.tile([P, DC, D], f32, tag="wq_f")
        nc.sync.dma_start(out=wq_f, in_=w_ext.rearrange("(c p) d -> p c d", p=P))
        wqT = singles.tile([P, DC, D], bf16, tag="wqT")
        for dcc in range(DC):
            for oc in range(DC):
                tp = ps_st.tile([P, P], f32, tag="st",
                                name=f"wqt_{dcc}_{oc}")
                nc.tensor.transpose(tp, wq_f[:, oc, dcc * P:(dcc + 1) * P],
                                    ident)
                nc.vector.tensor_copy(wqT[:, dcc, oc * P:(oc + 1) * P], tp)

        # ------------- load + transpose activations -------------
        # qT[p, dc, m] = query[m, dc*P + p]
        qT = singles.tile([P, DC, NQ], bf16, tag="qT")
        q_nat = q_ext.rearrange("(c p) d -> p c d", p=P)
        for c in range(NQ // (8 * P)):
            qf = fstage.tile([P, 8, D], f32, tag="fst")
            nc.sync.dma_start(out=qf, in_=q_nat[:, c * 8:(c + 1) * 8, :])
            for j in range(8):
                mc = c * 8 + j
                for dcc in range(DC):
                    tp = ps_st.tile([P, P], f32, tag="st",
                                    name=f"qt_{mc}_{dcc}")
                    nc.tensor.transpose(tp, qf[:, j, dcc * P:(dcc + 1) * P],
                                        ident)
                    nc.vector.tensor_copy(qT[:, dcc, mc * P:(mc + 1) * P], tp)

        # kT[p, dc, n] = key[n, dc*P + p]
        kT = singles.tile([P, DC, S], bf16, tag="kT")
        k_nat = k_ext.rearrange("(c p) d -> p c d", p=P)
        for c in range(S // (8 * P)):
            kf = fstage.tile([P, 8, D], f32, tag="fst")
            nc.sync.dma_start(out=kf, in_=k_nat[:, c * 8:(c + 1) * 8, :])
            for j in range(8):
                kc = c * 8 + j
                for dcc in range(DC):
                    tp = ps_st.tile([P, P], f32, tag="st",
                                    name=f"kt_{kc}_{dcc}")
                    nc.tensor.transpose(tp, kf[:, j, dcc * P:(dcc + 1) * P],
                                        ident)
                    nc.vector.tensor_copy(kT[:, dcc, kc * P:(kc + 1) * P], tp)

        # v natural bf16: v_bf[p, kc, d] = value[kc*P + p, d]
        v_bf = singles.tile([P, NKC, D], bf16, tag="v_bf")
        v_nat = v_ext.rearrange("(c p) d -> p c d", p=P)
        for c in range(S // (8 * P)):
            vf = fstage.tile([P, 8, D], f32, tag="fst")
            nc.sync.dma_start(out=vf, in_=v_nat[:, c * 8:(c + 1) * 8, :])
            nc.vector.tensor_copy(v_bf[:, c * 8:(c + 1) * 8, :], vf)

        # ------------- projections (transposed layouts) -------------
        # q_projT[p, oc, m] = q_proj[m, oc*P + p]
        q_pT = singles.tile([P, DC, NQ], bf16, tag="q_pT")
        for oc in range(DC):
            for msi in range(NSUP):
                pp = ps_st.tile([P, MSUP], f32, tag="st", name=f"qp_{oc}_{msi}")
                for dcc in range(DC):
                    nc.tensor.matmul(pp,
                                     lhsT=wqT[:, dcc, oc * P:(oc + 1) * P],
                                     rhs=qT[:, dcc, msi * MSUP:(msi + 1) * MSUP],
                                     start=(dcc == 0), stop=(dcc == DC - 1))
                nc.vector.tensor_scalar_add(
                    q_pT[:, oc, msi * MSUP:(msi + 1) * MSUP], pp,
                    bq_pc[:, oc:oc + 1])

        # k_projT[p, oc, n]
        k_pT = singles.tile([P, DC, S], bf16, tag="k_pT")
        for oc in range(DC):
            for nsi in range(S // MSUP):
                pp = ps_st.tile([P, MSUP], f32, tag="st", name=f"kp_{oc}_{nsi}")
                for dcc in range(DC):
                    nc.tensor.matmul(pp,
                                     lhsT=wqT[:, dcc, oc * P:(oc + 1) * P],
                                     rhs=kT[:, dcc, nsi * MSUP:(nsi + 1) * MSUP],
                                     start=(dcc == 0), stop=(dcc == DC - 1))
                nc.vector.tensor_scalar_add(
                    k_pT[:, oc, nsi * MSUP:(nsi + 1) * MSUP], pp,
                    bq_pc[:, oc:oc + 1])

        # ------------- attention main loop -------------
        for msi in range(NSUP):
            msl = slice(msi * MSUP, (msi + 1) * MSUP)
            ut_ps = ps_ut.tile([P, DC, MSUP], f32, tag="ut", name=f"ut_{msi}")
            s_ps = ps_epi.tile([P, MSUP], f32, tag="epi", name=f"s_{msi}")
            for g in range(NKC // GN):
                st_ps = ps_st.tile([P, GN, MSUP], f32, tag="st",
                                   name=f"st_{msi}_{g}")
                for i in range(GN):
                    kc = g * GN + i
                    for oc in range(DC):
                        nc.tensor.matmul(st_ps[:, i, :],
                                         lhsT=k_pT[:, oc, kc * P:(kc + 1) * P],
                                         rhs=q_pT[:, oc, msl],
                                         start=(oc == 0), stop=(oc == DC - 1),
                                         skip_group_check=True)
                pt = ptpool.tile([P, GN, MSUP], bf16, tag="pt",
                                 name=f"pt_{msi}_{g}")
                nc.scalar.activation(out=pt, in_=st_ps, func=FT.Exp, scale=SCALE)
                for i in range(GN):
                    kc = g * GN + i
                    for dcc in range(DC):
                        nc.tensor.matmul(ut_ps[:, dcc, :],
                                         lhsT=v_bf[:, kc, dcc * P:(dcc + 1) * P],
                                         rhs=pt[:, i, :],
                                         start=(kc == 0), stop=(kc == NKC - 1),
                                         skip_group_check=True)
                    nc.tensor.matmul(s_ps,
                                     lhsT=ones_mat, rhs=pt[:, i, :],
                                     start=(kc == 0), stop=(kc == NKC - 1),
                                     skip_group_check=True)

            # ---- epilogue for this m-super ----
            # s_ps rows are all identical (ones_mat) -> reciprocal directly
            recip_full = rsbp.tile([P, MSUP], f32, tag="rsb", name=f"rf_{msi}")
            nc.vector.reciprocal(recip_full, s_ps)
            # normalize U while casting to bf16
            ut_sb = utsbp.tile([P, DC, MSUP], bf16, tag="utsb",
                               name=f"utsb_{msi}")
            for dcc in range(DC):
                nc.vector.tensor_mul(ut_sb[:, dcc, :], ut_ps[:, dcc, :],
                                     recip_full)
            for mc in range(MSUP // P):
                o_ps = ps_epi.tile([P, D], f32, tag="epi", name=f"o_{msi}_{mc}")
                for dcc in range(DC):
                    nc.tensor.matmul(o_ps,
                                     lhsT=ut_sb[:, dcc, mc * P:(mc + 1) * P],
                                     rhs=wqT[:, dcc, :],
                                     start=(dcc == 0), stop=(dcc == DC - 1),
                                     skip_group_check=True)
                o_sb = osbp.tile([P, D], f32, tag="osb", name=f"osb_{msi}_{mc}")
                nc.vector.tensor_add(o_sb, o_ps, bq_full)
                row0 = msi * MSUP + mc * P
                nc.sync.dma_start(out=o_ext[row0:row0 + P, :], in_=o_sb)

    nc.finalize()
    return nc


def _get_nc():
    if "nc" not in _CACHE:
        _CACHE["nc"] = _build()
    return _CACHE["nc"]


def kernel(query, key, value, Wq, bq):
    from concourse.bass_utils import run_bass_kernel_spmd

    nc = _get_nc()
    in_maps = []
    for core in range(NCORES):
        b, h = core // 2, core % 2
        in_maps.append({
            "query": np.ascontiguousarray(query[b, h * NQ:(h + 1) * NQ, :],
                                          dtype=np.float32),
            "key": np.ascontiguousarray(key[b], dtype=np.float32),
            "value": np.ascontiguousarray(value[b], dtype=np.float32),
            "Wq": np.ascontiguousarray(Wq, dtype=np.float32),
            "bq": np.ascontiguousarray(bq, dtype=np.float32),
        })
    res = run_bass_kernel_spmd(nc, in_maps, core_ids=list(range(NCORES)))
    out = np.empty((B, S, D), np.float32)
    for core in range(NCORES):
        b, h = core // 2, core % 2
        out[b, h * NQ:(h + 1) * NQ, :] = res.results[core]["out"]
    return out



# revision 8
# speedup vs baseline: 1.8780x; 1.8780x over previous
"""Trainium2 Bass kernel for nn_AttentionHead_6365141532793.

Computes (per batch b):
    q = query @ Wq.T + bq ; k = key @ Wq.T + bq ; v = value @ Wq.T + bq
    out = softmax(q @ k.T / sqrt(D)) @ v

Sharding: 8 cores = 4 batches x 2 query-slabs (2048 rows each). Each core
holds the full key/value of its batch, so softmax rows are complete per
core and no collectives are needed.

Per-core algorithm (main-loop matmuls in fp8e4 with DoubleRow perf mode:
2 elem/cycle, 256-deep contraction folded into one matmul):
  - load Wq / q-slab / key as f32, cast bf16 (DVE), transpose 128x128
    blocks on the TensorEngine in bf16 (1 cyc/row vs 4 for f32):
    wqT[d, o] bf16, qT[d, m], kT[d, n]
  - load value, cast fp8e4 on the ScalarEngine: v[n, d]
  - q_projT[o, m] = wqT.T @ qT  (+bq fused in the PSUM->SBUF add, fp8 out)
  - k_projT[o, n] = wqT.T @ kT  (+bq, fp8 out)
  - for each m-super (512 query rows), for each pair of key chunks:
      scoresT[n, m] = k_projT.T @ q_projT     (1 DoubleRow mm per chunk)
      pT = exp(scoresT / 16)                  (ScalarE, [128,2,512], fp8)
      UT[d, m]   += v_pair.T @ pT             (DoubleRow, PSUM accum)
      s[*, m]    += ones.T @ pT               (row sums, replicated)
    The score matmul for group g+1 is issued before exp/UT of group g so
    the in-order PE queue never stalls on the ScalarEngine.
  - epilogue per m-super: recip = reciprocal_approx_fast(s);
    ut_sb = UT * recip (normalize while casting bf16);
    out[m, o] = ut_sb.T @ wqT + bq  (projection of U = P@V applied after
    attention: P @ (V Wq^T + 1 bq^T) = (P V) Wq^T + s bq^T)
No max-subtraction in softmax: scores/16 are O(1) for this distribution,
exp is safe and softmax is shift-invariant anyway.
"""

import contextlib
import dataclasses
import itertools

import numpy as np

B, S, D = 4, 4096, 256
NCORES = 8
NQ = S // 2          # query rows per core
P = 128
DC = D // P          # 2 chunks of the d/o dimension
MSUP = 512           # m-super: moving free dim per score matmul
NSUP = NQ // MSUP    # 4 m-supers per core
NKC = S // P         # 32 key chunks
GN = 2               # key chunks per group (DoubleRow pair)
NG = NKC // GN       # 16 groups
SCALE = 1.0 / 16.0   # 1/sqrt(D)

_CACHE = {}


def _build(num_devices=NCORES):
    import concourse.mybir as mybir
    import concourse.tile as tile
    from concourse import bacc
    from concourse.masks import make_identity

    f32 = mybir.dt.float32
    bf16 = mybir.dt.bfloat16
    fp8 = mybir.dt.float8e4
    FT = mybir.ActivationFunctionType
    DR = mybir.MatmulPerfMode.DoubleRow

    nc = bacc.Bacc("TRN2", target_bir_lowering=False, debug=False,
                   num_devices=num_devices)
    q_ext = nc.dram_tensor("query", [NQ, D], f32, kind="ExternalInput").ap()
    k_ext = nc.dram_tensor("key", [S, D], f32, kind="ExternalInput").ap()
    v_ext = nc.dram_tensor("value", [S, D], f32, kind="ExternalInput").ap()
    w_ext = nc.dram_tensor("Wq", [D, D], f32, kind="ExternalInput").ap()
    b_ext = nc.dram_tensor("bq", [D], f32, kind="ExternalInput").ap()
    o_ext = nc.dram_tensor("out", [NQ, D], f32, kind="ExternalOutput").ap()

    with contextlib.ExitStack() as ctx:
        tc = ctx.enter_context(tile.TileContext(nc))
        singles = ctx.enter_context(tc.tile_pool(name="singles", bufs=1))
        fstage = ctx.enter_context(tc.tile_pool(name="fstage", bufs=3))
        bstage = ctx.enter_context(tc.tile_pool(name="bstage", bufs=3))
        ptpool = ctx.enter_context(tc.tile_pool(name="ptpool", bufs=3))
        utsbp = ctx.enter_context(tc.tile_pool(name="utsbp", bufs=2))
        rsbp = ctx.enter_context(tc.tile_pool(name="rsbp", bufs=2))
        osbp = ctx.enter_context(tc.tile_pool(name="osbp", bufs=3))
        # PSUM budget (16KB/partition): st 2x4KB + ut 4KB + s 2KB + o 2x1KB
        ps_st = ctx.enter_context(tc.tile_pool(name="ps_st", bufs=2, space="PSUM"))
        ps_ut = ctx.enter_context(tc.tile_pool(name="ps_ut", bufs=1, space="PSUM"))
        ps_s = ctx.enter_context(tc.tile_pool(name="ps_s", bufs=1, space="PSUM"))
        ps_o = ctx.enter_context(tc.tile_pool(name="ps_o", bufs=1, space="PSUM"))

        # ---------------- constants ----------------
        ident_f = singles.tile([P, P], f32, tag="identf")
        make_identity(nc, ident_f)
        ident = singles.tile([P, P], bf16, tag="ident")
        nc.vector.tensor_copy(ident, ident_f)
        # full ones matrix: rowsum matmul -> s replicated on all partitions
        ones_f8 = singles.tile([P, GN, P], fp8, tag="ones")
        nc.vector.memset(ones_f8, 1.0)
        # bq_pc[p, oc] = bq[oc*P + p] (per-partition bias for projT layouts)
        bq_pc = singles.tile([P, DC], f32, tag="bq_pc")
        nc.sync.dma_start(out=bq_pc, in_=b_ext.rearrange("(c p) -> p c", p=P))
        # bq_full[p, o] = bq[o] -- broadcast across partitions (DMA step-0)
        bq_full = singles.tile([P, D], f32, tag="bq_full")
        b_bc = dataclasses.replace(
            b_ext, ap=[[0, P]] + [list(c) for c in b_ext.ap])
        nc.sync.dma_start(out=bq_full, in_=b_bc)

        # prologue PSUM: transposes from ps_st/ps_o, projections ps_ut/ps_s
        # (tags must match each pool's main-loop tag so regions are shared)
        tp_pools = itertools.cycle([(ps_st, "st"), (ps_o, "o")])
        pp_pools = itertools.cycle([(ps_ut, "ut"), (ps_s, "s")])

        # wqT[p, dc, o] = Wq[o, dc*P + p]  (bf16 PE transpose of bf16 cast)
        wq_f = fstage.tile([P, DC, D], f32, tag="fst", name="wq_f")
        nc.sync.dma_start(out=wq_f, in_=w_ext.rearrange("(c p) d -> p c d", p=P))
        wq_b = bstage.tile([P, DC, D], bf16, tag="bst", name="wq_b")
        nc.vector.tensor_copy(wq_b, wq_f)
        wqT = singles.tile([P, DC, D], bf16, tag="wqT")
        for dcc in range(DC):
            pool, tag = next(tp_pools)
            tp = pool.tile([P, DC, P], bf16, tag=tag, name=f"wqt_{dcc}")
            for oc in range(DC):
                nc.tensor.transpose(tp[:, oc, :],
                                    wq_b[:, oc, dcc * P:(dcc + 1) * P], ident)
            nc.vector.tensor_copy(wqT[:, dcc, :], tp)

        # ------------- load + cast + transpose activations -------------
        def load_transpose(ext, nrows, dst):
            nat = ext.rearrange("(c p) d -> p c d", p=P)
            for c in range(nrows // (8 * P)):
                tf = fstage.tile([P, 8, D], f32, tag="fst", name=f"f_{id(ext)}_{c}")
                nc.sync.dma_start(out=tf, in_=nat[:, c * 8:(c + 1) * 8, :])
                tb = bstage.tile([P, 8, D], bf16, tag="bst",
                                 name=f"b_{id(ext)}_{c}")
                nc.vector.tensor_copy(tb, tf)
                for dcc in range(DC):
                    for jh in range(2):
                        pool, tag = next(tp_pools)
                        tp = pool.tile([P, 4, P], bf16, tag=tag,
                                       name=f"t_{id(ext)}_{c}_{dcc}_{jh}")
                        for j4 in range(4):
                            j = jh * 4 + j4
                            nc.tensor.transpose(
                                tp[:, j4, :], tb[:, j, dcc * P:(dcc + 1) * P],
                                ident)
                        mc0 = c * 8 + jh * 4
                        nc.vector.tensor_copy(dst[:, dcc, mc0 * P:(mc0 + 4) * P],
                                              tp)
                yield c

        def project(srcT, dst, nsi):
            # dst[p, oc, nsi-slab] = (Wq srcT + bq)  in fp8
            nsl = slice(nsi * MSUP, (nsi + 1) * MSUP)
            for oc in range(DC):
                pool, tag = next(pp_pools)
                pp = pool.tile([P, MSUP], f32, tag=tag,
                               name=f"pp_{id(dst)}_{oc}_{nsi}")
                for dcc in range(DC):
                    nc.tensor.matmul(pp, lhsT=wqT[:, dcc, oc * P:(oc + 1) * P],
                                     rhs=srcT[:, dcc, nsl],
                                     start=(dcc == 0), stop=(dcc == DC - 1))
                nc.vector.tensor_scalar_add(dst[:, oc, nsl], pp,
                                            bq_pc[:, oc:oc + 1])

        # qT[p, dc, m] = query[m, dc*P + p] (bf16); q_pT fp8
        qT = singles.tile([P, DC, NQ], bf16, tag="qT")
        q_pT = singles.tile([P, DC, NQ], fp8, tag="q_pT")
        for c in load_transpose(q_ext, NQ, qT):
            for nsi in (2 * c, 2 * c + 1):
                project(qT, q_pT, nsi)

        # kT / k_pT over the full key range
        kT = singles.tile([P, DC, S], bf16, tag="kT")
        k_pT = singles.tile([P, DC, S], fp8, tag="k_pT")
        for c in load_transpose(k_ext, S, kT):
            for nsi in (2 * c, 2 * c + 1):
                project(kT, k_pT, nsi)

        # v natural fp8: v_f8[p, kc, d] = value[kc*P + p, d] (ScalarE cast)
        v_f8 = singles.tile([P, NKC, D], fp8, tag="v_f8")
        v_nat = v_ext.rearrange("(c p) d -> p c d", p=P)
        for c in range(S // (8 * P)):
            vf = fstage.tile([P, 8, D], f32, tag="fst", name=f"vf_{c}")
            nc.sync.dma_start(out=vf, in_=v_nat[:, c * 8:(c + 1) * 8, :])
            nc.scalar.copy(v_f8[:, c * 8:(c + 1) * 8, :], vf)

        # ------------- attention main loop (sw-pipelined) -------------
        uts, sps, sts = {}, {}, {}

        def epilogue(msi):
            ut_ps = uts.pop(msi)
            s_ps = sps.pop(msi)
            recip = rsbp.tile([P, MSUP], f32, tag="recip", name=f"r_{msi}")
            nc.vector.reciprocal_approx_fast(recip, s_ps)
            ut_sb = utsbp.tile([P, DC, MSUP], bf16, tag="utsb",
                               name=f"utsb_{msi}")
            for dcc in range(DC):
                nc.vector.tensor_mul(ut_sb[:, dcc, :], ut_ps[:, dcc, :], recip)
            for mc in range(MSUP // P):
                o_ps = ps_o.tile([P, D], f32, tag="o", name=f"o_{msi}_{mc}")
                for dcc in range(DC):
                    nc.tensor.matmul(o_ps,
                                     lhsT=ut_sb[:, dcc, mc * P:(mc + 1) * P],
                                     rhs=wqT[:, dcc, :],
                                     start=(dcc == 0), stop=(dcc == DC - 1),
                                     skip_group_check=True)
                o_sb = osbp.tile([P, D], f32, tag="osb", name=f"ob_{msi}_{mc}")
                nc.vector.tensor_add(o_sb, o_ps, bq_full)
                row0 = msi * MSUP + mc * P
                nc.sync.dma_start(out=o_ext[row0:row0 + P, :], in_=o_sb)

        TT = NSUP * NG
        for t in range(TT + 1):
            if t < TT:
                # issue score matmuls for group t (one group of lookahead)
                msi, g = divmod(t, NG)
                if g == 0:
                    uts[msi] = ps_ut.tile([P, DC, MSUP], f32, tag="ut",
                                          name=f"ut_{msi}")
                    sps[msi] = ps_s.tile([P, MSUP], f32, tag="s",
                                         name=f"s_{msi}")
                st = ps_st.tile([P, GN, MSUP], f32, tag="st", name=f"st_{t}")
                msl = slice(msi * MSUP, (msi + 1) * MSUP)
                for i in range(GN):
                    kc = g * GN + i
                    nc.tensor.matmul(st[:, i, :],
                                     lhsT=k_pT[:, :, kc * P:(kc + 1) * P],
                                     rhs=q_pT[:, :, msl],
                                     start=True, stop=True, perf_mode=DR,
                                     skip_group_check=True)
                sts[t] = st
            if t > 0:
                tp_ = t - 1
                msi, g = divmod(tp_, NG)
                st = sts.pop(tp_)
                pt = ptpool.tile([P, GN, MSUP], fp8, tag="pt", name=f"pt_{tp_}")
                nc.scalar.activation(out=pt, in_=st, func=FT.Exp, scale=SCALE)
                for dcc in range(DC):
                    nc.tensor.matmul(uts[msi][:, dcc, :],
                                     lhsT=v_f8[:, GN * g:GN * (g + 1),
                                               dcc * P:(dcc + 1) * P],
                                     rhs=pt,
                                     start=(g == 0), stop=(g == NG - 1),
                                     perf_mode=DR, skip_group_check=True)
                nc.tensor.matmul(sps[msi], lhsT=ones_f8, rhs=pt,
                                 start=(g == 0), stop=(g == NG - 1),
                                 perf_mode=DR, skip_group_check=True)
                if g == NG - 1:
                    epilogue(msi)

    nc.finalize()
    return nc


def _get_nc():
    if "nc" not in _CACHE:
        _CACHE["nc"] = _build()
    return _CACHE["nc"]


def kernel(query, key, value, Wq, bq):
    from concourse.bass_utils import run_bass_kernel_spmd

    nc = _get_nc()
    in_maps = []
    for core in range(NCORES):
        b, h = core // 2, core % 2
        in_maps.append({
            "query": np.ascontiguousarray(query[b, h * NQ:(h + 1) * NQ, :],
                                          dtype=np.float32),
            "key": np.ascontiguousarray(key[b], dtype=np.float32),
            "value": np.ascontiguousarray(value[b], dtype=np.float32),
            "Wq": np.ascontiguousarray(Wq, dtype=np.float32),
            "bq": np.ascontiguousarray(bq, dtype=np.float32),
        })
    res = run_bass_kernel_spmd(nc, in_maps, core_ids=list(range(NCORES)))
    out = np.empty((B, S, D), np.float32)
    for core in range(NCORES):
        b, h = core // 2, core % 2
        out[b, h * NQ:(h + 1) * NQ, :] = res.results[core]["out"]
    return out
